# revision 22
# baseline (speedup 1.0000x reference)
"""Trainium2 Bass kernel for a 3-layer dense transformer (BigramModel).

Contract: kernel(**inputs) takes the FULL unsharded numpy inputs (as produced
by setup_inputs) and returns the full [B, T, V] float32 logits. Internally the
batch dim B=128 is sharded 16-per-core across 8 NeuronCores (pure data
parallelism, weights replicated), one Bass/Tile NEFF run via
run_bass_kernel_spmd.

Layout strategy on device (per core, 16 seqs x 256 tok = 4096 tokens):
  - residual h: token-major fp32 SBUF tiles [128, 384] x 32 (persistent)
  - LayerNorm: DVE bn_stats/bn_aggr; rstd = exp(-0.5*ln(var+eps)); the
    normalize multiply runs on GpSimd; gamma/beta fold into adjacent weights.
  - all transposes (xn -> xnT, onorm -> oT) run on the PE (transpose via
    identity) with Vector/Scalar evacuation -- the XBAR DMA transpose
    serializes on the Sync engine (~1.2us each) and is never used.
  - matmuls in bf16 (fp32 PSUM accumulation).
  - attention: per (seq, head) scores kept feature-major [s, t] so softmax
    sums run through the matmul path: V is augmented with a ones column so
    the o-matmul also produces the softmax denominators; probs are masked
    multiplicatively after exp. o accumulates the two key-chunks in PSUM;
    evacuation fuses the 1/denominator multiply (per-head reciprocal).
  - proj and the MLP second linear produce TOKEN-major outputs directly
    (activation^T chunks as the stationary operand) so the residual add
    consumes PSUM straight -- no transpose back.
  - biases on free dims (bproj, b2, beta@Wv) are added inside PSUM via K=1
    ones-row matmuls, emitted only when the host sees nonzero values.
"""

import numpy as np
import ml_dtypes

BF16 = ml_dtypes.bfloat16

P = 128
T = 256
E = 384
V = 65
H = 6
HS = 64
FF = 1536
L = 3
NCORES = 8
BPC = 16              # sequences per core
TOK = BPC * T         # 4096 tokens per core
NT = TOK // P         # 32 token tiles
NB = TOK // 512       # 8 blocks of 512 tokens (2 seqs)
ECH = E // P          # 3
FCH = FF // P         # 12

_NC_CACHE = {}


def _build_nc(flags):
    """Build + compile the Bass program. flags = (bv_nz, bp_nz, b2_nz) per layer."""
    import concourse.bacc as bacc
    import concourse.mybir as mybir
    import concourse.tile as tile

    dt = mybir.dt
    f32 = dt.float32
    bf = dt.bfloat16
    Alu = mybir.AluOpType
    Act = mybir.ActivationFunctionType

    from concourse.masks import make_identity

    nc = bacc.Bacc("TRN2", target_bir_lowering=False, debug=False, num_devices=1)

    # ---- DRAM tensors (shapes match SBUF layouts; host pre-arranges) ----
    D = {}
    D["oh"] = nc.dram_tensor("oh", [V, TOK], bf, kind="ExternalInput")
    D["te"] = nc.dram_tensor("te", [V, E], bf, kind="ExternalInput")
    D["pos"] = nc.dram_tensor("pos", [P, 2, E], f32, kind="ExternalInput")
    D["mask"] = nc.dram_tensor("mask", [P, 2 * P], bf, kind="ExternalInput")
    for l in range(L):
        for w in ("wq", "wk", "wv", "wproj"):
            D[f"{w}{l}"] = nc.dram_tensor(f"{w}{l}", [P, ECH, E], bf, kind="ExternalInput")
        D[f"bq{l}"] = nc.dram_tensor(f"bq{l}", [P, ECH], f32, kind="ExternalInput")
        D[f"bk{l}"] = nc.dram_tensor(f"bk{l}", [P, ECH], f32, kind="ExternalInput")
        D[f"w1{l}"] = nc.dram_tensor(f"w1{l}", [P, ECH, FF], bf, kind="ExternalInput")
        D[f"b1c{l}"] = nc.dram_tensor(f"b1c{l}", [P, FCH], f32, kind="ExternalInput")
        D[f"w2{l}"] = nc.dram_tensor(f"w2{l}", [P, FCH, E], bf, kind="ExternalInput")
        D[f"bvrow{l}"] = nc.dram_tensor(f"bvrow{l}", [1, E], bf, kind="ExternalInput")
        D[f"bprow{l}"] = nc.dram_tensor(f"bprow{l}", [1, E], bf, kind="ExternalInput")
        D[f"b2row{l}"] = nc.dram_tensor(f"b2row{l}", [1, E], bf, kind="ExternalInput")
    D["wout"] = nc.dram_tensor("wout", [P, ECH, V], bf, kind="ExternalInput")
    D["boutc"] = nc.dram_tensor("boutc", [V, 1], f32, kind="ExternalInput")
    D["logT"] = nc.dram_tensor("logT", [V, TOK], f32, kind="ExternalOutput")

    bv_nz, bp_nz, b2_nz = flags

    with tile.TileContext(nc) as tc:
        import contextlib

        with contextlib.ExitStack() as ctx:
            const = ctx.enter_context(tc.tile_pool(name="const", bufs=1))
            wpool = ctx.enter_context(tc.tile_pool(name="wpool", bufs=2))
            act = ctx.enter_context(tc.tile_pool(name="act", bufs=4))
            act2 = ctx.enter_context(tc.tile_pool(name="act2", bufs=2))
            act1 = ctx.enter_context(tc.tile_pool(name="act1", bufs=1))
            ps_lin = ctx.enter_context(tc.tile_pool(name="ps_lin", bufs=5, space="PSUM"))
            ps_tp = ctx.enter_context(tc.tile_pool(name="ps_tp", bufs=3, space="PSUM"))

            def load_const(name, shape, dtp):
                t = const.tile(shape, dtp, tag=name)
                nc.sync.dma_start(out=t[:], in_=D[name].ap())
                return t

            # pad the K=65 embedding contraction to K=128 (sub-128 partition
            # matmuls are flaky on HW); pad rows are zeroed so they add 0.
            oh_sb = const.tile([P, TOK], bf, tag="oh")
            nc.vector.memset(oh_sb[:], 0.0)
            nc.sync.dma_start(out=oh_sb[0:V, :], in_=D["oh"].ap())
            te_sb = const.tile([P, E], bf, tag="te")
            nc.vector.memset(te_sb[:], 0.0)
            nc.sync.dma_start(out=te_sb[0:V, :], in_=D["te"].ap())
            pos_sb = load_const("pos", [P, 2, E], f32)
            mask_sb = load_const("mask", [P, 2 * P], bf)
            boutc_sb = load_const("boutc", [V, 1], f32)
            ones_sb = const.tile([1, P], bf, tag="ones")
            nc.vector.memset(ones_sb[:], 1.0)
            eps_sb = const.tile([P, 1], f32, tag="eps")
            nc.vector.memset(eps_sb[:], 1e-5)
            ident_sb = const.tile([P, P], bf, tag="ident")
            make_identity(nc, ident_sb[:])
            _tp_ctr = [0]

            def tpose(dst, src):
                """dst[P,128] (sbuf bf16) = transpose(src[P,128] sbuf bf16),
                via the PE; evacuation rotates 3:1 over Vector/Scalar."""
                tp = ps_tp.tile([P, P], bf, tag="tp", name="tp")
                nc.tensor.transpose(tp[:], src, ident_sb[:])
                k = _tp_ctr[0] = _tp_ctr[0] + 1
                if k % 4 == 3:
                    nc.scalar.copy(out=dst, in_=tp[:])
                else:
                    nc.vector.tensor_copy(out=dst, in_=tp[:])

            # persistent residual tiles
            h = []
            for i in range(NT):
                h.append(const.tile([P, E], f32, tag=f"h{i}", name=f"h{i}"))

            # ---- embedding: h = onehot.T @ tok_emb + pos ----
            for i in range(NT):
                ps = ps_lin.tile([P, E], f32, tag="mm")
                nc.tensor.matmul(
                    ps[:], oh_sb[:, i * P:(i + 1) * P], te_sb[:],
                    start=True, stop=True,
                )
                nc.vector.tensor_add(out=h[i][:], in0=ps[:], in1=pos_sb[:, i % 2, :])

            def ln_block(i0):
                """LN of h[i0..i0+3] -> xnT bf16 [P,ECH,512] (feature-major)."""
                xn = act2.tile([P, 4, E], bf, tag="xn")
                mv4 = act.tile([P, 4, 2], f32, tag="mv")
                rstd4 = act.tile([P, 4], f32, tag="rstd")

                def norm_j(j):
                    nc.vector.tensor_scalar(
                        out=xn[:, j, :], in0=h[i0 + j][:],
                        scalar1=mv4[:, j, 0:1], scalar2=rstd4[:, j:j + 1],
                        op0=Alu.subtract, op1=Alu.mult,
                    )

                # rstd = exp(-0.5 * ln(var + eps)), PER TILE so tile j's
                # normalize never waits on tile 3's stats; each normalize is
                # emitted one tile behind its stats so the DVE queue never
                # idles on the ACT rstd hop. Cuts first-transpose latency
                # from ~4.5us to ~1.5us.
                for j in range(4):
                    st6 = act.tile([P, 6], f32, tag="bnst")
                    nc.vector.bn_stats(out=st6[:], in_=h[i0 + j][:])
                    nc.vector.bn_aggr(out=mv4[:, j, :], in_=st6[:])
                    nc.scalar.activation(
                        out=rstd4[:, j:j + 1], in_=mv4[:, j, 1:2],
                        func=Act.Ln, bias=eps_sb[:],
                    )
                    nc.scalar.activation(
                        out=rstd4[:, j:j + 1], in_=rstd4[:, j:j + 1],
                        func=Act.Exp, scale=-0.5,
                    )
                    if j > 0:
                        norm_j(j - 1)
                norm_j(3)
                xnT = act.tile([P, ECH, 512], bf, tag="xnT")
                for j in range(4):
                    for c in range(ECH):
                        tpose(
                            xnT[:, c, j * P:(j + 1) * P],
                            xn[:, j, c * P:(c + 1) * P],
                        )
                return xnT

            def linear_fmaj(xnT, w_sb, bias_sb, fch, tag, relu=False,
                            act_evac=False):
                """feature-major out [P, fch, 512] bf16 = (W^T xn^T);
                bias per-partition. relu/act_evac route evac to ScalarE."""
                o = (act1 if fch == FCH else act2).tile([P, fch, 512], bf, tag=tag, name=tag)
                for f in range(fch):
                    ps = ps_lin.tile([P, 512], f32, tag="mm")
                    for c in range(ECH):
                        nc.tensor.matmul(
                            ps[:], w_sb[:, c, f * P:(f + 1) * P], xnT[:, c, :],
                            start=(c == 0), stop=(c == ECH - 1),
                        )
                    if relu:
                        nc.scalar.activation(
                            out=o[:, f, :], in_=ps[:], func=Act.Relu,
                            bias=bias_sb[:, f:f + 1], scale=1.0,
                        )
                    elif act_evac:
                        nc.scalar.activation(
                            out=o[:, f, :], in_=ps[:], func=Act.Identity,
                            bias=bias_sb[:, f:f + 1], scale=1.0,
                        )
                    else:
                        nc.vector.tensor_scalar_add(
                            out=o[:, f, :], in0=ps[:], scalar1=bias_sb[:, f:f + 1],
                        )
                return o

            def linear_tok_resid(xT, w_sb, nch, brow, i0):
                """h[i0+j] += x @ W (+ b): token-major PSUM output via xT
                chunks as the stationary operand; residual add reads PSUM."""
                for j in range(4):
                    ps = ps_lin.tile([P, E], f32, tag="mm", name="tokmm")
                    for c in range(nch):
                        nc.tensor.matmul(
                            ps[:], xT[:, c, j * P:(j + 1) * P], w_sb[:, c, :],
                            start=(c == 0),
                            stop=(c == nch - 1 and brow is None),
                        )
                    if brow is not None:
                        nc.tensor.matmul(
                            ps[:], ones_sb[:], brow[:], start=False, stop=True,
                        )
                    nc.vector.tensor_add(
                        out=h[i0 + j][:], in0=h[i0 + j][:], in1=ps[:])

            def load_w(name, shape, dtp):
                t = wpool.tile(shape, dtp, tag=name[:-1])  # tag without layer idx
                nc.sync.dma_start(out=t[:], in_=D[name].ap())
                return t

            # ---- transformer layers (software-pipelined emission) ----
            W = {}

            def load_layer(l):
                W[l] = dict(
                    wq=load_w(f"wq{l}", [P, ECH, E], bf),
                    wk=load_w(f"wk{l}", [P, ECH, E], bf),
                    wv=load_w(f"wv{l}", [P, ECH, E], bf),
                    wproj=load_w(f"wproj{l}", [P, ECH, E], bf),
                    bq=load_w(f"bq{l}", [P, ECH], f32),
                    bk=load_w(f"bk{l}", [P, ECH], f32),
                    w1=load_w(f"w1{l}", [P, ECH, FF], bf),
                    b1c=load_w(f"b1c{l}", [P, FCH], f32),
                    w2=load_w(f"w2{l}", [P, FCH, E], bf),
                    bvrow=load_w(f"bvrow{l}", [1, E], bf) if bv_nz[l] else None,
                    bprow=load_w(f"bprow{l}", [1, E], bf) if bp_nz[l] else None,
                    b2row=load_w(f"b2row{l}", [1, E], bf) if b2_nz[l] else None,
                )

            def attn_emit(l, b, xnT):
                Wl = W[l]
                wq, wk, wv = Wl["wq"], Wl["wk"], Wl["wv"]
                wproj, bq, bk = Wl["wproj"], Wl["bq"], Wl["bk"]
                bvrow, bprow = Wl["bvrow"], Wl["bprow"]
                i0 = 4 * b
                QT = linear_fmaj(xnT, wq, bq, ECH, "QT")
                KT = linear_fmaj(xnT, wk, bk, ECH, "KT")
                # V token-major, ones-augmented: [P, 4, H, 65]
                Vt = act2.tile([P, 4, H, 65], bf, tag="Vt")
                for j in range(4):
                    ps = ps_lin.tile([P, E], f32, tag="mm")
                    for c in range(ECH):
                        nc.tensor.matmul(
                            ps[:], xnT[:, c, j * P:(j + 1) * P], wv[:, c, :],
                            start=(c == 0),
                            stop=(c == ECH - 1 and bvrow is None),
                        )
                    if bvrow is not None:
                        nc.tensor.matmul(
                            ps[:], ones_sb[:], bvrow[:], start=False, stop=True,
                        )
                    nc.vector.tensor_copy(
                        out=Vt[:, j, :, 0:64],
                        in_=ps.rearrange("p (h d) -> p h d", h=H),
                    )
                    nc.gpsimd.memset(Vt[:, j, :, 64:65], 1.0)

                oT = act2.tile([P, ECH, 512], bf, tag="oT")
                # pass 1: scores/probs for BOTH seqs, so each seq's exp+mask
                # chain completes under the other seq's score matmuls.
                probs_l = []
                for s in range(2):      # the 2 sequences in this block
                    tb = s * 256        # col offset within the 512 block
                    probs = act2.tile([P, 2, H, 256], bf, tag="probs")
                    probs_l.append(probs)
                    for st in range(2):  # s_tile (128 keys each)
                        tlo = 128 if st == 1 else 0
                        for hh in range(H):
                            c, off = divmod(hh * HS, P)
                            sc = ps_lin.tile([P, 512], f32, tag="mm", name="sc")
                            nc.tensor.matmul(
                                sc[:, 0:256 - tlo],
                                KT[off:off + HS, c, tb + st * P: tb + (st + 1) * P],
                                QT[off:off + HS, c, tb + tlo: tb + 256],
                                start=True, stop=True,
                            )
                            nc.scalar.activation(
                                out=probs[:, st, hh, tlo:256],
                                in_=sc[:, 0:256 - tlo],
                                func=Act.Exp, scale=float(HS) ** -0.5,
                            )
                        if st == 0:
                            nc.vector.tensor_tensor(
                                out=probs[:, 0], in0=probs[:, 0],
                                in1=mask_sb[:, None, :].to_broadcast((P, H, 256)),
                                op=Alu.mult,
                            )
                        else:
                            nc.vector.tensor_tensor(
                                out=probs[:, 1, :, P:256],
                                in0=probs[:, 1, :, P:256],
                                in1=mask_sb[:, None, 0:P].to_broadcast((P, H, P)),
                                op=Alu.mult,
                            )
                # pass 2: o-matmuls, normalization, oT transposes.
                # All heads go into one [P, H, 65] PSUM tile (free-dim
                # offsets verified exact on HW); the two key chunks
                # accumulate in PSUM; evac fuses the softmax normalization.
                for s in range(2):
                    probs = probs_l[s]
                    onorm = act2.tile([P, 2, E], bf, tag="onorm")
                    for tt in range(2):  # query tiles of this seq
                        osum = ps_lin.tile([P, H, 65], f32, tag="mm", name="osum")
                        for hh in range(H):
                            nc.tensor.matmul(
                                osum[:, hh, :],
                                probs[:, 0, hh, tt * P:(tt + 1) * P],
                                Vt[:, 2 * s, hh, :],
                                start=True, stop=(tt == 0),
                            )
                            if tt == 1:
                                nc.tensor.matmul(
                                    osum[:, hh, :],
                                    probs[:, 1, hh, P:2 * P],
                                    Vt[:, 2 * s + 1, hh, :],
                                    start=False, stop=True,
                                )
                        rec = act.tile([P, H], f32, tag="rec", name="rec")
                        nc.vector.reciprocal(out=rec[:], in_=osum[:, :, 64])
                        nc.vector.tensor_tensor(
                            out=onorm[:, tt].rearrange("p (h d) -> p h d", h=H),
                            in0=osum[:, :, 0:64],
                            in1=rec[:, :, None].to_broadcast((P, H, HS)),
                            op=Alu.mult,
                        )
                    for tt in range(2):
                        for c in range(ECH):
                            tpose(
                                oT[:, c, (2 * s + tt) * P:(2 * s + tt + 1) * P],
                                onorm[:, tt, c * P:(c + 1) * P],
                            )
                linear_tok_resid(oT, wproj, ECH, bprow, i0)

            def mlp_emit(l, b):
                i0 = 4 * b
                xnT2 = ln_block(i0)
                aT = linear_fmaj(xnT2, W[l]["w1"], W[l]["b1c"], FCH, "aT",
                                 relu=True)
                linear_tok_resid(aT, W[l]["w2"], FCH, W[l]["b2row"], i0)

            wout = wpool.tile([P, ECH, V], bf, tag="wout")
            nc.sync.dma_start(out=wout[:], in_=D["wout"].ap())

            def final_emit(b, xnfT):
                ps = ps_lin.tile([V, 512], f32, tag="mm")
                for c in range(ECH):
                    nc.tensor.matmul(
                        ps[:], wout[:, c, :], xnfT[:, c, :],
                        start=(c == 0), stop=(c == ECH - 1),
                    )
                lt = act2.tile([V, 512], f32, tag="lt")
                nc.vector.tensor_scalar_add(out=lt[:], in0=ps[:], scalar1=boutc_sb[:])
                nc.sync.dma_start(
                    out=D["logT"].ap()[:, b * 512:(b + 1) * 512], in_=lt[:],
                )

            # stage pipeline: LN for stage i+1 is emitted during stage i,
            # and stage i's MLP trails one stage behind its attention, so
            # the DVE/ACT LayerNorm chains hide under PE-heavy stretches.
            load_layer(0)
            if L > 1:
                load_layer(1)
            stages = [(l, b) for l in range(L) for b in range(NB)]
            stages += [(L, b) for b in range(NB)]      # final LN + unembed
            xnT_pre = ln_block(0)
            for idx, (l, b) in enumerate(stages):
                if l < L:
                    attn_emit(l, b, xnT_pre)
                else:
                    final_emit(b, xnT_pre)
                if idx > 0 and stages[idx - 1][0] < L:
                    pl, pb = stages[idx - 1]
                    mlp_emit(pl, pb)
                    if pb == NB - 1 and pl + 2 < L:
                        load_layer(pl + 2)
                # LN for the next stage is emitted last: its rstd Ln/Exp
                # pulls the ACT function table back to the exp set AFTER the
                # trailing MLP's relus, so the next stage's probs exps don't
                # eat the table-swap latency.
                if idx + 1 < len(stages):
                    xnT_pre = ln_block(4 * stages[idx + 1][1])
            pl, pb = stages[-1]
            if pl < L:
                mlp_emit(pl, pb)

    nc.compile()
    return nc


def _prep_shared(inp):
    """Host-side weight prep: layout rearrangement + LN gamma/beta folding."""
    sh = {}

    def f32(x):
        return np.asarray(x, np.float32)

    sh["te"] = np.asarray(f32(inp["tok_emb"]), BF16)                      # [V,E]
    sh["pos"] = np.ascontiguousarray(
        f32(inp["pos_emb"]).reshape(2, P, E).transpose(1, 0, 2))          # [P,2,E]
    m = np.concatenate(
        [np.triu(np.ones((P, P), np.float32)), np.ones((P, P), np.float32)], axis=1)
    sh["mask"] = np.asarray(m, BF16)                                      # [P,256]

    def tile3(w, fdim):  # [E, fdim] -> [P, ECH, fdim]
        return np.ascontiguousarray(w.reshape(ECH, P, fdim).transpose(1, 0, 2))

    def col(b, nch):  # [nch*P] -> [P, nch]
        return np.ascontiguousarray(b.reshape(nch, P).T)

    bv_nz, bp_nz, b2_nz = [], [], []
    for l in range(L):
        g1, b1_ = f32(inp["ln1_g"][l]), f32(inp["ln1_b"][l])
        g2, b2_ = f32(inp["ln2_g"][l]), f32(inp["ln2_b"][l])
        wq = f32(inp["Wq"][l]).transpose(1, 0, 2).reshape(E, E)   # head-major cols
        wk = f32(inp["Wk"][l]).transpose(1, 0, 2).reshape(E, E)
        wv = f32(inp["Wv"][l]).transpose(1, 0, 2).reshape(E, E)
        sh[f"wq{l}"] = np.asarray(tile3(g1[:, None] * wq, E), BF16)
        sh[f"wk{l}"] = np.asarray(tile3(g1[:, None] * wk, E), BF16)
        sh[f"wv{l}"] = np.asarray(tile3(g1[:, None] * wv, E), BF16)
        sh[f"bq{l}"] = col(wq.T @ b1_, ECH)
        sh[f"bk{l}"] = col(wk.T @ b1_, ECH)
        bv = wv.T @ b1_
        sh[f"bvrow{l}"] = np.asarray(bv[None, :], BF16)
        bv_nz.append(bool(np.any(bv != 0)))
        wp = f32(inp["Wproj"][l])
        sh[f"wproj{l}"] = np.asarray(tile3(wp, E), BF16)
        bp = f32(inp["bproj"][l])
        sh[f"bprow{l}"] = np.asarray(bp[None, :], BF16)
        bp_nz.append(bool(np.any(bp != 0)))
        w1 = f32(inp["W1"][l])
        sh[f"w1{l}"] = np.asarray(tile3(g2[:, None] * w1, FF), BF16)
        sh[f"b1c{l}"] = col(f32(inp["b1"][l]) + w1.T @ b2_, FCH)
        w2 = f32(inp["W2"][l])
        sh[f"w2{l}"] = np.asarray(
            w2.reshape(FCH, P, E).transpose(1, 0, 2), BF16)
        b2r = f32(inp["b2"][l])
        sh[f"b2row{l}"] = np.asarray(b2r[None, :], BF16)
        b2_nz.append(bool(np.any(b2r != 0)))

    gf, bf_ = f32(inp["lnf_g"]), f32(inp["lnf_b"])
    wo = f32(inp["Wout"])
    sh["wout"] = np.asarray(tile3(gf[:, None] * wo, V), BF16)
    sh["boutc"] = (f32(inp["bout"]) + wo.T @ bf_).reshape(V, 1)
    flags = (tuple(bv_nz), tuple(bp_nz), tuple(b2_nz))
    return sh, flags


def _onehot(xc):
    """xc: [BPC, T] ints -> [V, TOK] bf16 one-hot (feature-major)."""
    xf = np.asarray(xc, np.int64).reshape(-1)
    oh = np.zeros((V, TOK), np.float32)
    oh[xf, np.arange(TOK)] = 1.0
    return np.asarray(oh, BF16)


def _get_nc(flags):
    if flags not in _NC_CACHE:
        _NC_CACHE[flags] = _build_nc(flags)
    return _NC_CACHE[flags]


def make_in_maps(inputs):
    sh, flags = _prep_shared(inputs)
    x = np.asarray(inputs["x"])
    in_maps = []
    for c in range(NCORES):
        m = dict(sh)
        m["oh"] = _onehot(x[c * BPC:(c + 1) * BPC])
        in_maps.append(m)
    return in_maps, flags


def kernel(**inputs):
    import os
    from concourse.bass_utils import run_bass_kernel_spmd

    in_maps, flags = make_in_maps(inputs)
    nc = _get_nc(flags)
    kw = {}
    if os.environ.get("BASS_TRACE"):
        d = os.environ.get("BASS_TRACE_DIR", "/tmp/bass_trace")
        os.makedirs(d, exist_ok=True)
        kw["tmpdir"] = d
    res = run_bass_kernel_spmd(nc, in_maps, list(range(NCORES)), **kw)
    kernel._last = res
    outs = []
    for c in range(NCORES):
        lt = np.asarray(res.results[c]["logT"], np.float32)   # [V, TOK]
        outs.append(np.ascontiguousarray(lt.T).reshape(BPC, T, V))
    return np.concatenate(outs, axis=0)


kernel._last = None
